# revision 12
# baseline (speedup 1.0000x reference)
"""BrainAgeGAT Trainium2 kernel: 2-layer GATv2 + mean-pool + MLP on 8 NeuronCores.

Strategy (per sharding_hint: shard edges; 1D-shard nodes; all-reduce pool):
  - Edges (incl. self loops) are sorted by destination and sharded by
    destination-node range across the 8 cores, so each core owns the full
    softmax/scatter for its destination nodes.
  - Per-core destination nodes are packed into blocks of <=127 "slots"
    (slot 127 of each 128-row block is a garbage slot).
  - Node transforms xl = x@Wl / xr = x@Wr are computed on each core for its
    own node shard; the xl table is AllGather'd so every core can gather any
    source row. Per edge, 512-byte bf16 rows are fetched with dma_gather
    (SWDGE gather, int16 indices; the 51200-row global table is split in two
    halves to stay within int16).
  - Per-tile one-hot matrices (edge->slot, and transposed) are precomputed
    on the host and DMA'd from DRAM; the transposed one-hot expands per-block
    xr rows to per-edge xr via a PE matmul that accumulates with an
    identity-matmul of the gathered xl rows, yielding u = xl[src]+xr[dst]
    directly in PSUM (no per-edge xr gather, no on-device one-hot build).
  - logits = per-head tree-reduction of att * leaky_relu(u) (ACT + DVE);
    softmax needs no max subtraction at these magnitudes. The scatter
    message is exp(logit)*xl[src] (one-hot matmul accumulated in PSUM) and
    the bias is added per destination node at the end.
  - Mean-pool uses per-block one-hot graph-selector matmuls into a
    persistent PSUM accumulator, an 8-core AllReduce, and a tiny MLP.
"""

import math
import sys

sys.path.insert(0, "/opt/trn_rl_repo")

import ml_dtypes
import numpy as np

import concourse.bacc as bacc
import concourse.bass as bass
import concourse.mybir as mybir
import concourse.tile as tile
from concourse import library_config
from concourse.vector_clock import ScopedClock

BF16 = ml_dtypes.bfloat16

# ---------------------------------------------------------------------------
# Patches for walrus' one-sync-wait-per-instruction limit.
# ---------------------------------------------------------------------------


def _drain_and_barrier(self, tick_clock, wait_clock):
    nc = self.nc
    probe = nc.sync.nop(nofuse=True, hint="drain_wait_split")
    wait_clock.add_sem_waits(probe.ins, ScopedClock({None: tick_clock.global_clock}))
    si = probe.ins.sync_info
    waits = list(si.on_wait) if si and si.on_wait else []
    if len(waits) > 1:
        si.on_wait = waits[:1]
        for w in waits[1:]:
            extra = nc.sync.nop(nofuse=True, hint="drain_wait_split")
            extra.ins.sync_info = type(si)(on_wait=[w], on_update=[])
    nc.sync.drain()
    nc.all_engine_barrier()
    assert self.sems is not None
    popped = nc._tile_sem_poison_stack.pop()
    assert popped is self._sem_poison
    nc.clear_and_free_semaphores(list(self.sems.allocated().values()))
    nc.all_engine_barrier()


tile.TileContext._drain_and_barrier = _drain_and_barrier


def _split_waits(nc):
    """walrus codegen accepts one sync-wait command per instruction; Tile can
    emit several. Hoist extras onto preceding same-engine NoOps."""
    for bb in nc.main_func.blocks:
        out = []
        for ins in bb.instructions:
            si = ins.sync_info
            waits = list(si.on_wait) if si and si.on_wait else []
            if len(waits) > 1:
                for w in waits[:-1]:
                    nop = mybir.InstNoOp(
                        name=nc.get_next_instruction_name(), ins=[], outs=[]
                    )
                    nop.engine = ins.engine
                    nop.sync_info = mybir.SyncInfo(on_wait=[w], on_update=[])
                    nc.register_instruction(nop)
                    out.append(nop)
                si.on_wait = [waits[-1]]
            out.append(ins)
        bb.instructions = out


# ---------------------------------------------------------------------------
# Model dimensions (hardcoded per problem spec)
# ---------------------------------------------------------------------------
N = 50000
E = 800000
G = 128
H = 8
C = 32
HC = H * C  # 256
P = 128
NCORES = 8
SLOTS = 127  # real slots per block (slot 127 = garbage)
MAXI16 = 25600  # table-piece size for int16 gather indices
CT = 4  # tiles per PSUM u-chunk


class Cfg:
    def __init__(self, n_nodes, ncores, nblk, tba, tbb):
        self.n_nodes = n_nodes
        self.ncores = ncores
        self.nodes_pc = n_nodes // ncores
        self.nblk = nblk
        self.cap = nblk * P
        self.capext = ncores * self.cap
        self.tba = tba  # list[nblk]
        self.tbb = tbb  # list[nblk]
        self.tb = [a + b for a, b in zip(tba, tbb)]
        self.ttot = sum(self.tb)
        self.col0 = np.concatenate([[0], np.cumsum(self.tb)]).astype(int)
        self.npiece = 2 if self.capext > MAXI16 else 1
        if self.npiece == 1:
            assert all(b == 0 for b in tbb)


# ---------------------------------------------------------------------------
# Host-side preprocessing
# ---------------------------------------------------------------------------


def _f32(a):
    return np.ascontiguousarray(a, dtype=np.float32)


def _bf(a):
    return np.ascontiguousarray(np.asarray(a, dtype=np.float32).astype(BF16))


def _i2mat():
    i2 = np.zeros((2 * P, HC), np.float32)
    for j in range(2):
        i2[j * P + np.arange(P), 2 * np.arange(P) + j] = 1.0
    return _bf(i2)


def _wrap_idx(ids):
    """Gather-index list -> [128, len/16] int16 in the SWDGE wrap layout."""
    ids = np.asarray(ids, np.int16)
    assert len(ids) % 16 == 0
    w = ids.reshape(-1, 16).T  # [16, s]
    return np.tile(w, (8, 1))  # [128, s]


def _plan_blocks(edge_index, n_nodes, ncores):
    """Sort/pad edges; return per-core edge structures + uniform tile counts."""
    npc = n_nodes // ncores
    nblk = (npc + SLOTS - 1) // SLOTS
    cap = nblk * P
    capext = ncores * cap
    npiece = 2 if capext > MAXI16 else 1

    src = np.concatenate([edge_index[0], np.arange(n_nodes)]).astype(np.int64)
    dst = np.concatenate([edge_index[1], np.arange(n_nodes)]).astype(np.int64)
    order = np.argsort(dst, kind="stable")
    src, dst = src[order], dst[order]

    sloc = src % npc
    srow = (src // npc) * cap + (sloc // SLOTS) * P + (sloc % SLOTS)

    percore = []
    na = np.zeros((ncores, nblk), int)
    nb_ = np.zeros((ncores, nblk), int)
    for c in range(ncores):
        lo = c * npc
        sel = (dst >= lo) & (dst < lo + npc)
        bsrow = srow[sel]
        loc = dst[sel] - lo
        blocks = []
        for b in range(nblk):
            es = (loc // SLOTS) == b
            rs = bsrow[es]
            slots = (loc[es] % SLOTS).astype(np.int64)
            piece = rs // MAXI16 if npiece == 2 else np.zeros_like(rs)
            a_i = np.where(piece == 0)[0]
            b_i = np.where(piece == 1)[0]
            blocks.append((rs, slots, a_i, b_i))
            na[c, b] = len(a_i)
            nb_[c, b] = len(b_i)
        percore.append(blocks)
    tba = [int(math.ceil((na[:, b].max() + 1) / P)) for b in range(nblk)]
    tbb = [int(math.ceil(nb_[:, b].max() / P)) if npiece == 2 else 0
           for b in range(nblk)]
    return percore, nblk, tba, tbb


def _prep(x, edge_index, batch, u, weights, cfg: Cfg, percore):
    npc = cfg.nodes_pc
    att1 = weights["att1"]
    att2 = weights["att2"]

    def att_rep(att):
        return _bf(np.broadcast_to(att.reshape(-1), (P, HC)))

    idx_cols_a = [t * 8 for t in cfg.tba]
    idx_cols_b = [t * 8 for t in cfg.tbb]

    maps = []
    for c in range(cfg.ncores):
        m = {}
        lo = c * npc
        ixa = np.zeros((P, sum(idx_cols_a)), np.int16)
        ixb = np.zeros((P, max(sum(idx_cols_b), 1)), np.int16)
        # per-edge slot ids in tile-major layout; -1 = pad (zero one-hot)
        slotv = np.full((cfg.ttot * P,), -1, np.int64)
        ca = cb = 0
        for b in range(cfg.nblk):
            rs, slots, a_i, b_i = percore[c][b]
            garb_ext = c * cfg.cap + b * P + 127
            na, nb_ = len(a_i), len(b_i)
            ea = cfg.tba[b] * P
            eb = cfg.tbb[b] * P
            ia = np.full(ea, garb_ext % MAXI16, np.int64)
            ia[:na] = rs[a_i] % MAXI16
            if garb_ext >= MAXI16:
                ia[na:] = 0
            ib = np.full(eb, 0, np.int64)
            ib[:nb_] = rs[b_i] % MAXI16
            ixa[:, ca : ca + cfg.tba[b] * 8] = _wrap_idx(ia)
            if eb:
                ixb[:, cb : cb + cfg.tbb[b] * 8] = _wrap_idx(ib)
            off = np.full(ea + eb, -1, np.int64)
            off[:na] = slots[a_i]
            off[ea : ea + nb_] = slots[b_i]
            slotv[cfg.col0[b] * P : cfg.col0[b + 1] * P] = off
            ca += cfg.tba[b] * 8
            cb += cfg.tbb[b] * 8
        m["ixa"] = ixa
        m["ixb"] = ixb
        # one-hot tables: edge position p of tile g is slotv[g*128+p]
        sv = slotv.reshape(cfg.ttot, P)  # [g, p] (p = edge pos in tile)
        ar = np.arange(P)
        F8 = ml_dtypes.float8_e4m3fn
        # Otd[p, g*128+s] = (sv[g, p] == s)
        ot = (sv[:, :, None] == ar[None, None, :])  # [g, p, s]
        m["Otd"] = np.ascontiguousarray(
            ot.transpose(1, 0, 2).reshape(P, cfg.ttot * P).astype(F8))
        # OtTd[s, g*128+e] = (sv[g, e] == s)
        m["OtTd"] = np.ascontiguousarray(
            ot.transpose(2, 0, 1).reshape(P, cfg.ttot * P).astype(F8))

        xs = np.zeros((cfg.cap, x.shape[1]), np.float32)
        rows = (np.arange(npc) // SLOTS) * P + (np.arange(npc) % SLOTS)
        xs[rows] = x[lo : lo + npc]
        m["xT"] = _bf(xs.T)

        gsel = np.zeros((cfg.cap, G), np.float32)
        gsel[rows, np.asarray(batch[lo : lo + npc])] = 1.0
        m["gsel"] = _bf(gsel)
        maps.append(m)

    counts = np.bincount(np.asarray(batch), minlength=G).astype(np.float32)
    shared = {
        "Wl1": _bf(weights["Wl1"]),
        "Wr1": _bf(weights["Wr1"]),
        "Wl2": _bf(weights["Wl2"]),
        "Wr2": _bf(weights["Wr2"]),
        "att1r": att_rep(att1),
        "att2r": att_rep(att2),
        "b1r": _bf(np.broadcast_to(weights["b1"], (P, HC))),
        "b2r": _bf(np.broadcast_to(weights["b2"], (P, HC))),
        "ident": _bf(np.eye(P, dtype=np.float32)),
        "i2": _i2mat(),
        "crecip": _f32((1.0 / np.maximum(counts, 1.0)).reshape(G, 1)),
        "Wlin1": _bf(weights["W_lin1"]),
        "blin1r": _f32(np.broadcast_to(weights["b_lin1"], (G, 64))),
        "Wout": _bf(weights["W_out"]),
        "boutr": _f32(np.full((G, 1), float(weights["b_out"][0]), np.float32)),
        "ub": _bf(u),
    }
    for m in maps:
        m.update(shared)
    return maps


# ---------------------------------------------------------------------------
# Device program
# ---------------------------------------------------------------------------


def _build(cfg: Cfg, in_dim=3):
    dt = mybir.dt
    bf = dt.bfloat16
    f32 = dt.float32
    nc = bacc.Bacc(None)
    groups = [list(range(cfg.ncores))]

    def prm(name, shape, dtype):
        return nc.declare_dram_parameter(name, list(shape), dtype, isOutput=False)

    xT = prm("xT", [in_dim, cfg.cap], bf)
    ixa = prm("ixa", [P, sum(t * 8 for t in cfg.tba)], dt.int16)
    ixb = prm("ixb", [P, max(sum(t * 8 for t in cfg.tbb), 1)], dt.int16)
    Otd = prm("Otd", [P, cfg.ttot * P], dt.float8e4)
    OtTd = prm("OtTd", [P, cfg.ttot * P], dt.float8e4)
    Wl1p = prm("Wl1", [in_dim, HC], bf)
    Wr1p = prm("Wr1", [in_dim, HC], bf)
    Wl2p = prm("Wl2", [HC, HC], bf)
    Wr2p = prm("Wr2", [HC, HC], bf)
    att1r = prm("att1r", [P, HC], bf)
    att2r = prm("att2r", [P, HC], bf)
    b1r = prm("b1r", [P, HC], bf)
    b2r = prm("b2r", [P, HC], bf)
    identp = prm("ident", [P, P], bf)
    i2p = prm("i2", [2 * P, HC], bf)
    gselp = prm("gsel", [cfg.cap, G], bf)
    crecip = prm("crecip", [G, 1], f32)
    Wlin1 = prm("Wlin1", [HC, 64], bf)
    blin1r = prm("blin1r", [G, 64], f32)
    Woutp = prm("Wout", [64 + 3, 1], bf)
    boutr = prm("boutr", [G, 1], f32)
    ub = prm("ub", [G, 3], bf)
    out_g = nc.declare_dram_parameter("out_g", [G, 1], f32, isOutput=True)

    with tile.TileContext(nc) as tc:
        with (
            tc.tile_pool(name="const", bufs=1) as constp,
            tc.tile_pool(name="tab", bufs=1) as tabp,
            tc.tile_pool(name="meta", bufs=2) as metap,
            tc.tile_pool(name="gbuf", bufs=1) as gbufp,
            tc.tile_pool(name="work", bufs=2) as workp,
            tc.tile_pool(name="small", bufs=3) as smallp,
            tc.tile_pool(name="psA", bufs=1, space="PSUM") as psA,
            tc.tile_pool(name="psB", bufs=2, space="PSUM") as psB,
            tc.tile_pool(name="psU", bufs=2, space="PSUM") as psU,
            tc.tile_pool(name="psG", bufs=1, space="PSUM") as psG,
            tc.tile_pool(name="dram", bufs=1, space="DRAM") as dram,
        ):
            # ---- constants to SBUF ----
            def cload(p):
                t = constp.tile([p.shape[0], p.shape[1]], p.dtype, name=p.name + "_s")
                nc.sync.dma_start(out=t[:], in_=p[:])
                return t

            def cload_k(p):
                nk = (p.shape[0] + P - 1) // P
                out = []
                for kt in range(nk):
                    rows = slice(kt * P, min((kt + 1) * P, p.shape[0]))
                    t = constp.tile(
                        [rows.stop - rows.start, p.shape[1]], p.dtype,
                        name=f"{p.name}_s{kt}",
                    )
                    nc.sync.dma_start(out=t[:], in_=p[rows, :])
                    out.append(t)
                return out

            xT_s = cload(xT)
            Wl1_s = cload_k(Wl1p)
            Wr1_s = cload_k(Wr1p)
            Wl2_s = cload_k(Wl2p)
            Wr2_s = cload_k(Wr2p)
            att1r_s = cload(att1r)
            att2r_s = cload(att2r)
            b1r_s = cload(b1r)
            b2r_s = cload(b2r)
            ident_s = cload(identp)
            i2_s = []
            for j in range(2):
                t = constp.tile([P, HC], bf, name=f"i2_s{j}")
                nc.sync.dma_start(out=t[:], in_=i2p[j * P : (j + 1) * P, :])
                i2_s.append(t)
            crecip_s = cload(crecip)
            Wlin1_s = cload_k(Wlin1)
            blin1r_s = cload(blin1r)
            Wout_s = cload(Woutp)
            boutr_s = cload(boutr)
            ub_s = cload(ub)

            # ---- internal DRAM ----
            # xl tables live transposed+interleaved: [128, n, 2] with
            # [i, n, j] = xl[n, j*128+i], so the SBUF gather table loads as
            # one big contiguous DMA per core chunk.
            xl1_ownT = dram.tile([P, cfg.cap * 2], bf)
            xr1_tab = dram.tile([cfg.cap, HC], bf)
            xrb1 = dram.tile([cfg.cap, HC], bf)
            xl1_extT = dram.tile([cfg.ncores * P, cfg.cap * 2], bf, addr_space="Shared")
            h1T = dram.tile([2, P, cfg.cap], bf)
            xl2_ownT = dram.tile([P, cfg.cap * 2], bf)
            xr2_tab = dram.tile([cfg.cap, HC], bf)
            xrb2 = dram.tile([cfg.cap, HC], bf)
            xl2_extT = dram.tile([cfg.ncores * P, cfg.cap * 2], bf, addr_space="Shared")
            gp_in = dram.tile([G, HC], f32)
            gp_out = dram.tile([G, HC], f32, addr_space="Shared")

            A_ = mybir.AluOpType
            AF = mybir.ActivationFunctionType

            # ================= node tables =================
            def node_tables(lhsT_tiles, Wl_s, Wr_s, br_s, xlT_dst, xr_dst, xrb_dst):
                for b in range(cfg.nblk):
                    rows = slice(b * P, (b + 1) * P)
                    for W_s, kind in ((Wl_s, "l"), (Wr_s, "r")):
                        ps = psA.tile([P, HC], f32, tag="a")
                        lts = lhsT_tiles(b)
                        assert len(lts) == len(W_s)
                        for i, lt in enumerate(lts):
                            nc.tensor.matmul(
                                ps[:], lhsT=lt, rhs=W_s[i][:],
                                start=(i == 0), stop=(i == len(lts) - 1),
                            )
                        ev = smallp.tile([P, HC], bf, tag="tabev")
                        nc.scalar.activation(out=ev[:], in_=ps[:], func=AF.Copy)
                        if kind == "r":
                            nc.sync.dma_start(out=xr_dst[rows, :], in_=ev[:])
                            xb = smallp.tile([P, HC], bf, tag="tabxb")
                            nc.vector.tensor_tensor(
                                out=xb[:], in0=br_s[:], in1=ps[:], op=A_.subtract
                            )
                            nc.sync.dma_start(out=xrb_dst[rows, :], in_=xb[:])
                        else:
                            # transpose + 2x interleave in one pass: permuted
                            # identities place column n of chunk j at 2n+j.
                            tp = psA.tile([P, HC], f32, tag="a")
                            for j in range(2):
                                nc.tensor.matmul(
                                    tp[:], lhsT=ev[:, j * P : (j + 1) * P],
                                    rhs=i2_s[j][:], start=(j == 0), stop=(j == 1),
                                )
                            st = smallp.tile([P, HC], bf, tag="xlT")
                            nc.scalar.activation(out=st[:], in_=tp[:], func=AF.Copy)
                            nc.sync.dma_start(
                                out=xlT_dst[:, rows.start * 2 : rows.stop * 2],
                                in_=st[:],
                            )

            node_tables(
                lambda b: [xT_s[:, b * P : (b + 1) * P]],
                Wl1_s, Wr1_s, b1r_s, xl1_ownT, xr1_tab, xrb1,
            )
            nc.gpsimd.collective_compute(
                "AllGather", A_.bypass, replica_groups=groups,
                ins=[xl1_ownT.opt()], outs=[xl1_extT.opt()],
            )

            # ================= edge pipeline =================
            # Two passes per layer: piece A (source cores 0-3) then piece B
            # (cores 4-7). The 12.8MB xl table piece for the pass lives in
            # SBUF ([128, 25600, 2] bf16) and per-edge rows come from
            # ap_gather (GPSIMD SBUF gather, no DMA descriptors). Gathered
            # rows are feature-major; PE transposes them into the PSUM
            # u-accumulator (xl part is copied out to SBUF for the message
            # before the xr one-hot expansion accumulates on top). Pass A
            # scatter sums are staged to SBUF and chained into pass B's
            # accumulator with an identity matmul.
            CH = 5  # tiles per ap_gather call

            def edge_layer(xl_extT, xr_tab, xrb_tab, attr_s, layer):
                gpool_ps = None
                if layer == 2:
                    gpool_ps = psG.tile([G, HC], f32, name=f"gpool_ps{layer}")
                accA = dram.tile([cfg.nblk * P, HC + H], bf, name=f"accA{layer}")
                for pc in range(cfg.npiece):
                    tabS = tabp.tile([P, MAXI16, 2], bf, tag="tab")
                    for ci in range(4):
                        nc.sync.dma_start(
                            out=tabS[:, ci * cfg.cap : (ci + 1) * cfg.cap, :]
                            .rearrange("p n j -> p (n j)"),
                            in_=xl_extT[(pc * 4 + ci) * P : (pc * 4 + ci + 1) * P, :],
                        )
                    ci_off = 0
                    for b in range(cfg.nblk):
                        tba, tbb = cfg.tba[b], cfg.tbb[b]
                        npt = tba if pc == 0 else tbb
                        t0 = 0 if pc == 0 else tba
                        c0 = cfg.col0[b]
                        xr_blk = metap.tile([P, HC], bf, tag="xrblk")
                        nc.sync.dma_start(
                            out=xr_blk[:], in_=xr_tab[b * P : (b + 1) * P, :]
                        )
                        if npt:
                            Ot_blk = metap.tile([P, npt, P], dt.float8e4, tag="Ot")
                            nc.sync.dma_start(
                                out=Ot_blk[:].rearrange("p t s -> p (t s)"),
                                in_=Otd[:, (c0 + t0) * P : (c0 + t0 + npt) * P],
                            )
                            OtT_blk = metap.tile([P, npt, P], dt.float8e4, tag="OtT")
                            nc.sync.dma_start(
                                out=OtT_blk[:].rearrange("p t e -> p (t e)"),
                                in_=OtTd[:, (c0 + t0) * P : (c0 + t0 + npt) * P],
                            )
                            ixp = ixa if pc == 0 else ixb
                            ix_t = metap.tile([P, npt * 8], dt.int16, tag="ix")
                            nc.sync.dma_start(
                                out=ix_t[:], in_=ixp[:, ci_off : ci_off + npt * 8]
                            )
                            gxl2 = gbufp.tile([P, npt * P, 2], bf, tag="gxl")
                            for q0 in range(0, npt, CH):
                                q1 = min(q0 + CH, npt)
                                nc.gpsimd.ap_gather(
                                    out_ap=gxl2[:, q0 * P : q1 * P, :],
                                    in_ap=tabS[:],
                                    idxs_ap=ix_t[:, q0 * 8 : q1 * 8],
                                    channels=P, num_elems=MAXI16, d=2,
                                    num_idxs=(q1 - q0) * P,
                                )

                            # u = xr[dst] + xl[src] on PE (region start by the
                            # xr one-hot expansion, transposes accumulate)
                            ft = workp.tile([P, npt, HC], bf, tag="ft", bufs=1)
                            for q0 in range(0, npt, CT):
                                q1 = min(q0 + CT, npt)
                                ut_ps = psU.tile([P, CT, HC], f32, tag="ut")
                                for t in range(q0, q1):
                                    nc.tensor.matmul(
                                        ut_ps[:, t - q0, :],
                                        lhsT=OtT_blk[:, t, :], rhs=xr_blk[:],
                                        start=True, stop=False,
                                        skip_group_check=True,
                                    )
                                    nc.tensor.matmul(
                                        ut_ps[:, t - q0, 0:P],
                                        lhsT=gxl2[:, t * P : (t + 1) * P, 0],
                                        rhs=ident_s[:], start=False, stop=True,
                                        skip_group_check=True,
                                    )
                                    nc.tensor.matmul(
                                        ut_ps[:, t - q0, P:HC],
                                        lhsT=gxl2[:, t * P : (t + 1) * P, 1],
                                        rhs=ident_s[:], start=False, stop=True,
                                        skip_group_check=True,
                                    )
                                nc.scalar.activation(
                                    out=ft[:, q0:q1, :],
                                    in_=ut_ps[:, 0 : q1 - q0, :],
                                    func=AF.Prelu, alpha=0.2,
                                )
                            # recover u = leaky_relu^-1(ft) before ft is
                            # overwritten by the attention multiply
                            u_sb = workp.tile([P, npt, HC], bf, tag="usb", bufs=1)
                            nc.scalar.activation(
                                out=u_sb[:], in_=ft[:], func=AF.Prelu, alpha=5.0
                            )

                            nc.vector.tensor_tensor(
                                out=ft[:], in0=ft[:],
                                in1=_bcast_mid(attr_s[:], npt), op=A_.mult,
                            )
                            v = ft[:].rearrange("p t (h c) -> p (t h) c", h=H)
                            t1 = workp.tile([P, npt * H, 16], bf, tag="t1", bufs=1)
                            nc.vector.tensor_tensor(out=t1[:], in0=v[:, :, 0:16], in1=v[:, :, 16:32], op=A_.add)
                            t2 = workp.tile([P, npt * H, 8], bf, tag="t2", bufs=1)
                            nc.vector.tensor_tensor(out=t2[:], in0=t1[:, :, 0:8], in1=t1[:, :, 8:16], op=A_.add)
                            t3 = workp.tile([P, npt * H, 4], bf, tag="t3", bufs=1)
                            nc.vector.tensor_tensor(out=t3[:], in0=t2[:, :, 0:4], in1=t2[:, :, 4:8], op=A_.add)
                            t4 = workp.tile([P, npt * H, 2], bf, tag="t4", bufs=1)
                            nc.vector.tensor_tensor(out=t4[:], in0=t3[:, :, 0:2], in1=t3[:, :, 2:4], op=A_.add)
                            lg = workp.tile([P, npt * H], bf, tag="lg", bufs=1)
                            nc.vector.tensor_tensor(
                                out=lg[:].unsqueeze(2), in0=t4[:, :, 0:1], in1=t4[:, :, 1:2], op=A_.add
                            )
                            msg = workp.tile([P, npt, HC + H], bf, tag="msg")
                            lgv = lg[:].rearrange("p (t h) -> p t h", h=H)
                            nc.scalar.activation(
                                out=msg[:, :, 0:HC].rearrange(
                                    "p t (h c) -> p t h c", h=H),
                                in_=lgv.unsqueeze(3).to_broadcast([P, npt, H, C]),
                                func=AF.Exp,
                            )
                            nc.vector.tensor_copy(
                                out=msg[:, :, HC : HC + H],
                                in_=msg[:, :, 0:HC].rearrange(
                                    "p t (h c) -> p t h c", h=H)[:, :, :, 0],
                            )
                            nc.vector.tensor_tensor(
                                out=msg[:, :, 0:HC], in0=msg[:, :, 0:HC],
                                in1=u_sb[:], op=A_.mult,
                            )
                            ci_off += npt * 8

                        acc = psB.tile([P, HC + H], f32, tag="b")
                        if pc == 0:
                            for t in range(npt):
                                nc.tensor.matmul(
                                    acc[:], lhsT=Ot_blk[:, t, :], rhs=msg[:, t, :],
                                    start=(t == 0), stop=(t == npt - 1),
                                )
                            ast = smallp.tile([P, HC + H], bf, tag="accst")
                            nc.scalar.activation(out=ast[:], in_=acc[:], func=AF.Copy)
                            nc.sync.dma_start(
                                out=accA[b * P : (b + 1) * P, :], in_=ast[:]
                            )
                            continue

                        # pass B: chain pass-A sums, accumulate B tiles, finish
                        xrb_blk = metap.tile([P, HC], bf, tag="xrbblk")
                        nc.sync.dma_start(
                            out=xrb_blk[:], in_=xrb_tab[b * P : (b + 1) * P, :]
                        )
                        accA_t = metap.tile([P, HC + H], bf, tag="accld")
                        nc.sync.dma_start(
                            out=accA_t[:], in_=accA[b * P : (b + 1) * P, :]
                        )
                        nc.tensor.matmul(
                            acc[:], lhsT=ident_s[:], rhs=accA_t[:],
                            start=True, stop=(npt == 0),
                        )
                        for t in range(npt):
                            nc.tensor.matmul(
                                acc[:], lhsT=Ot_blk[:, t, :], rhs=msg[:, t, :],
                                start=False, stop=(t == npt - 1),
                            )

                        denom = smallp.tile([P, H], f32, tag="denom")
                        nc.vector.tensor_scalar(
                            out=denom[:], in0=acc[:, HC : HC + H], scalar1=1e-20,
                            scalar2=None, op0=A_.max,
                        )
                        rec = smallp.tile([P, H], f32, tag="rec")
                        nc.vector.reciprocal(out=rec[:], in_=denom[:])
                        hsc = smallp.tile([P, HC], bf, tag="hsc")
                        nc.vector.tensor_tensor(
                            out=hsc[:].rearrange("p (h c) -> p h c", h=H),
                            in0=acc[:, 0:HC].rearrange("p (h c) -> p h c", h=H),
                            in1=rec[:].to_broadcast([P, H, C]),
                            op=A_.mult,
                        )
                        hfin = smallp.tile([P, HC], bf, tag="hfin")
                        nc.vector.tensor_tensor(out=hfin[:], in0=hsc[:], in1=xrb_blk[:], op=A_.add)
                        hout = smallp.tile([P, HC], bf, tag="hout")
                        nc.scalar.activation(out=hout[:], in_=hfin[:], func=AF.Relu)

                        if layer == 1:
                            for kt in range(2):
                                tp = psA.tile([P, P], bf, tag="a")
                                nc.tensor.transpose(
                                    out=tp[:], in_=hout[:, kt * P : (kt + 1) * P],
                                    identity=ident_s[:],
                                )
                                tps = smallp.tile([P, P], bf, tag="htps")
                                nc.scalar.activation(out=tps[:], in_=tp[:], func=AF.Copy)
                                nc.sync.dma_start(
                                    out=h1T[kt, :, b * P : (b + 1) * P], in_=tps[:]
                                )
                        else:
                            gsel_blk = metap.tile([P, G], bf, tag="gselb")
                            nc.sync.dma_start(
                                out=gsel_blk[:], in_=gselp[b * P : (b + 1) * P, :]
                            )
                            nc.tensor.matmul(
                                gpool_ps[:], lhsT=gsel_blk[:], rhs=hout[:],
                                start=(b == 0), stop=(b == cfg.nblk - 1),
                            )
                return gpool_ps

            edge_layer(xl1_extT, xr1_tab, xrb1, att1r_s, layer=1)

            # ================= layer-2 node tables =================
            def h1_lhsT(b):
                outs = []
                for kt in range(2):
                    t = smallp.tile([P, P], bf, tag="h1l", name=f"h1l{b}_{kt}")
                    nc.sync.dma_start(out=t[:], in_=h1T[kt, :, b * P : (b + 1) * P])
                    outs.append(t[:])
                return outs

            node_tables(h1_lhsT, Wl2_s, Wr2_s, b2r_s, xl2_ownT, xr2_tab, xrb2)
            nc.gpsimd.collective_compute(
                "AllGather", A_.bypass, replica_groups=groups,
                ins=[xl2_ownT.opt()], outs=[xl2_extT.opt()],
            )

            gpool_ps = edge_layer(xl2_extT, xr2_tab, xrb2, att2r_s, layer=2)

            # ================= pool + MLP =================
            gsum = smallp.tile([G, HC], f32, tag="gsum")
            nc.scalar.activation(out=gsum[:], in_=gpool_ps[:], func=AF.Copy)
            nc.sync.dma_start(out=gp_in[:], in_=gsum[:])
            nc.gpsimd.collective_compute(
                "AllReduce", A_.add, replica_groups=groups,
                ins=[gp_in.opt()], outs=[gp_out.opt()],
            )
            gsum2 = smallp.tile([G, HC], f32, tag="gsum2")
            nc.sync.dma_start(out=gsum2[:], in_=gp_out[:])
            gmean = smallp.tile([G, HC], bf, tag="gmean")
            nc.vector.tensor_scalar(
                out=gmean[:], in0=gsum2[:], scalar1=crecip_s[:, 0:1], scalar2=None,
                op0=A_.mult,
            )
            gT = []
            for kt in range(2):
                tp = psA.tile([P, G], bf, tag="a")
                nc.tensor.transpose(
                    out=tp[:], in_=gmean[:, kt * P : (kt + 1) * P], identity=ident_s[:]
                )
                gkt = smallp.tile([P, G], bf, tag="gT", name=f"gT{kt}")
                nc.scalar.activation(out=gkt[:], in_=tp[:], func=AF.Copy)
                gT.append(gkt)
            lin_ps = psB.tile([G, 64], f32, tag="b")
            for kt in range(2):
                nc.tensor.matmul(
                    lin_ps[:], lhsT=gT[kt][:], rhs=Wlin1_s[kt][:],
                    start=(kt == 0), stop=(kt == 1),
                )
            lin = smallp.tile([G, 64], f32, tag="lin")
            nc.vector.tensor_tensor(out=lin[:], in0=lin_ps[:], in1=blin1r_s[:], op=A_.add)
            glu = smallp.tile([G, P], bf, tag="glu")
            nc.scalar.activation(out=glu[:, 0:64], in_=lin[:], func=AF.Relu)
            nc.vector.tensor_copy(out=glu[:, 64:67], in_=ub_s[:])
            # rows 67..127 of gluT are never read by the final matmul
            tp = psA.tile([P, G], bf, tag="a")
            nc.tensor.transpose(out=tp[:], in_=glu[:], identity=ident_s[:])
            gluT = smallp.tile([P, G], bf, tag="gluT")
            nc.scalar.activation(out=gluT[:], in_=tp[:], func=AF.Copy)
            out_ps = psB.tile([G, 1], f32, tag="b")
            nc.tensor.matmul(
                out_ps[:], lhsT=gluT[0:67, :], rhs=Wout_s[:], start=True, stop=True
            )
            outs = smallp.tile([G, 1], f32, tag="outs")
            nc.vector.tensor_tensor(out=outs[:], in0=out_ps[:], in1=boutr_s[:], op=A_.add)
            nc.sync.dma_start(out=out_g[:], in_=outs[:])

    nc.compile()
    _split_waits(nc)
    return nc


def _bcast_mid(ap, reps):
    return ap.unsqueeze(1).broadcast_to([ap.shape[0], reps, ap.shape[1]])


# ---------------------------------------------------------------------------
# Entry point
# ---------------------------------------------------------------------------


def kernel(**inputs):
    import os

    from concourse.bass_utils import run_bass_kernel_spmd

    x = np.asarray(inputs["x"], np.float32)
    edge_index = np.asarray(inputs["edge_index"], np.int64)
    batch = np.asarray(inputs["batch"], np.int64)
    u = np.asarray(inputs["u"], np.float32)
    weights = {
        k: np.asarray(inputs[k], np.float32)
        for k in ("Wl1", "Wr1", "att1", "b1", "Wl2", "Wr2", "att2", "b2",
                  "W_lin1", "b_lin1", "W_out", "b_out")
    }
    percore, nblk, tba, tbb = _plan_blocks(edge_index, N, NCORES)
    cfg = Cfg(N, NCORES, nblk, tba, tbb)
    maps = _prep(x, edge_index, batch, u, weights, cfg, percore)
    nc = _build(cfg, in_dim=x.shape[1])
    trace = bool(os.environ.get("KERNEL_TRACE"))
    kw = {}
    if trace:
        tmpdir = os.environ.get("KERNEL_TRACE_DIR", "/tmp/ktrace")
        os.makedirs(tmpdir, exist_ok=True)
        kw["tmpdir"] = tmpdir
    try:
        res = run_bass_kernel_spmd(nc, maps, list(range(NCORES)), trace=trace, **kw)
    except ModuleNotFoundError:
        res = run_bass_kernel_spmd(nc, maps, list(range(NCORES)))
    if trace and getattr(res, "exec_time_ns", None) is not None:
        print(f"HW exec time: {res.exec_time_ns} ns")
        if res.instructions_and_trace is not None:
            print(f"trace: {res.instructions_and_trace[1]}")
    return res.results[0]["out_g"].reshape(G).astype(np.float32)


# revision 14
# speedup vs baseline: 2.3185x; 2.3185x over previous
"""BrainAgeGAT Trainium2 kernel: 2-layer GATv2 + mean-pool + MLP on 8 NeuronCores.

Strategy (per sharding_hint: shard edges; 1D-shard nodes; all-reduce pool):
  - Edges (incl. self loops) are sorted by destination and sharded by
    destination-node range across the 8 cores, so each core owns the full
    softmax/scatter for its destination nodes.
  - Per-core destination nodes are packed into blocks of <=127 "slots"
    (slot 127 of each 128-row block is a garbage slot).
  - Node transforms xl = x@Wl / xr = x@Wr are computed on each core for its
    own node shard; the xl table is AllGather'd so every core can gather any
    source row. Per edge, 512-byte bf16 rows are fetched with dma_gather
    (SWDGE gather, int16 indices; the 51200-row global table is split in two
    halves to stay within int16).
  - Per-tile one-hot matrices (edge->slot, and transposed) are precomputed
    on the host and DMA'd from DRAM; the transposed one-hot expands per-block
    xr rows to per-edge xr via a PE matmul that accumulates with an
    identity-matmul of the gathered xl rows, yielding u = xl[src]+xr[dst]
    directly in PSUM (no per-edge xr gather, no on-device one-hot build).
  - logits = per-head tree-reduction of att * leaky_relu(u) (ACT + DVE);
    softmax needs no max subtraction at these magnitudes. The scatter
    message is exp(logit)*xl[src] (one-hot matmul accumulated in PSUM) and
    the bias is added per destination node at the end.
  - Mean-pool uses per-block one-hot graph-selector matmuls into a
    persistent PSUM accumulator, an 8-core AllReduce, and a tiny MLP.
"""

import math
import sys

sys.path.insert(0, "/opt/trn_rl_repo")

import ml_dtypes
import numpy as np

import concourse.bacc as bacc
import concourse.bass as bass
import concourse.mybir as mybir
import concourse.tile as tile
from concourse import library_config
from concourse.vector_clock import ScopedClock

BF16 = ml_dtypes.bfloat16

# ---------------------------------------------------------------------------
# Patches for walrus' one-sync-wait-per-instruction limit.
# ---------------------------------------------------------------------------


def _drain_and_barrier(self, tick_clock, wait_clock):
    nc = self.nc
    probe = nc.sync.nop(nofuse=True, hint="drain_wait_split")
    wait_clock.add_sem_waits(probe.ins, ScopedClock({None: tick_clock.global_clock}))
    si = probe.ins.sync_info
    waits = list(si.on_wait) if si and si.on_wait else []
    if len(waits) > 1:
        si.on_wait = waits[:1]
        for w in waits[1:]:
            extra = nc.sync.nop(nofuse=True, hint="drain_wait_split")
            extra.ins.sync_info = type(si)(on_wait=[w], on_update=[])
    nc.sync.drain()
    nc.all_engine_barrier()
    assert self.sems is not None
    popped = nc._tile_sem_poison_stack.pop()
    assert popped is self._sem_poison
    nc.clear_and_free_semaphores(list(self.sems.allocated().values()))
    nc.all_engine_barrier()


tile.TileContext._drain_and_barrier = _drain_and_barrier


def _split_waits(nc):
    """walrus codegen accepts one sync-wait command per instruction; Tile can
    emit several. Hoist extras onto preceding same-engine NoOps."""
    for bb in nc.main_func.blocks:
        out = []
        for ins in bb.instructions:
            si = ins.sync_info
            waits = list(si.on_wait) if si and si.on_wait else []
            if len(waits) > 1:
                for w in waits[:-1]:
                    nop = mybir.InstNoOp(
                        name=nc.get_next_instruction_name(), ins=[], outs=[]
                    )
                    nop.engine = ins.engine
                    nop.sync_info = mybir.SyncInfo(on_wait=[w], on_update=[])
                    nc.register_instruction(nop)
                    out.append(nop)
                si.on_wait = [waits[-1]]
            out.append(ins)
        bb.instructions = out


# ---------------------------------------------------------------------------
# Model dimensions (hardcoded per problem spec)
# ---------------------------------------------------------------------------
N = 50000
E = 800000
G = 128
H = 8
C = 32
HC = H * C  # 256
P = 128
NCORES = 8
SLOTS = 127  # real slots per block (slot 127 = garbage)
MAXI16 = 25600  # table-piece size for int16 gather indices
CT = 4  # tiles per PSUM u-chunk


class Cfg:
    def __init__(self, n_nodes, ncores, nblk, tba, tbb):
        self.n_nodes = n_nodes
        self.ncores = ncores
        self.nodes_pc = n_nodes // ncores
        self.nblk = nblk
        self.cap = nblk * P
        self.capext = ncores * self.cap
        self.tba = tba  # list[nblk]
        self.tbb = tbb  # list[nblk]
        self.tb = [a + b for a, b in zip(tba, tbb)]
        self.ttot = sum(self.tb)
        self.col0 = np.concatenate([[0], np.cumsum(self.tb)]).astype(int)
        self.npiece = 2 if self.capext > MAXI16 else 1
        if self.npiece == 1:
            assert all(b == 0 for b in tbb)


# ---------------------------------------------------------------------------
# Host-side preprocessing
# ---------------------------------------------------------------------------


def _f32(a):
    return np.ascontiguousarray(a, dtype=np.float32)


def _bf(a):
    return np.ascontiguousarray(np.asarray(a, dtype=np.float32).astype(BF16))


def _wrap_idx(ids):
    """Gather-index list -> [128, len/16] int16 in the SWDGE wrap layout."""
    ids = np.asarray(ids, np.int16)
    assert len(ids) % 16 == 0
    w = ids.reshape(-1, 16).T  # [16, s]
    return np.tile(w, (8, 1))  # [128, s]


def _plan_blocks(edge_index, n_nodes, ncores):
    """Sort/pad edges; return per-core edge structures + uniform tile counts."""
    npc = n_nodes // ncores
    nblk = (npc + SLOTS - 1) // SLOTS
    cap = nblk * P
    capext = ncores * cap
    npiece = 2 if capext > MAXI16 else 1

    src = np.concatenate([edge_index[0], np.arange(n_nodes)]).astype(np.int64)
    dst = np.concatenate([edge_index[1], np.arange(n_nodes)]).astype(np.int64)
    order = np.argsort(dst, kind="stable")
    src, dst = src[order], dst[order]

    sloc = src % npc
    srow = (src // npc) * cap + (sloc // SLOTS) * P + (sloc % SLOTS)

    percore = []
    na = np.zeros((ncores, nblk), int)
    nb_ = np.zeros((ncores, nblk), int)
    for c in range(ncores):
        lo = c * npc
        sel = (dst >= lo) & (dst < lo + npc)
        bsrow = srow[sel]
        loc = dst[sel] - lo
        blocks = []
        for b in range(nblk):
            es = (loc // SLOTS) == b
            rs = bsrow[es]
            slots = (loc[es] % SLOTS).astype(np.int64)
            piece = rs // MAXI16 if npiece == 2 else np.zeros_like(rs)
            a_i = np.where(piece == 0)[0]
            b_i = np.where(piece == 1)[0]
            blocks.append((rs, slots, a_i, b_i))
            na[c, b] = len(a_i)
            nb_[c, b] = len(b_i)
        percore.append(blocks)
    tba = [int(math.ceil((na[:, b].max() + 1) / P)) for b in range(nblk)]
    tbb = [int(math.ceil(nb_[:, b].max() / P)) if npiece == 2 else 0
           for b in range(nblk)]
    return percore, nblk, tba, tbb


def _prep(x, edge_index, batch, u, weights, cfg: Cfg, percore):
    npc = cfg.nodes_pc
    att1 = weights["att1"]
    att2 = weights["att2"]

    def att_rep(att):
        return _bf(np.broadcast_to(att.reshape(-1), (P, HC)))

    idx_cols_a = [t * 8 for t in cfg.tba]
    idx_cols_b = [t * 8 for t in cfg.tbb]

    maps = []
    for c in range(cfg.ncores):
        m = {}
        lo = c * npc
        ixa = np.zeros((P, sum(idx_cols_a)), np.int16)
        ixb = np.zeros((P, max(sum(idx_cols_b), 1)), np.int16)
        # per-edge slot ids in tile-major layout; -1 = pad (zero one-hot)
        slotv = np.full((cfg.ttot * P,), -1, np.int64)
        ca = cb = 0
        for b in range(cfg.nblk):
            rs, slots, a_i, b_i = percore[c][b]
            garb_ext = c * cfg.cap + b * P + 127
            na, nb_ = len(a_i), len(b_i)
            ea = cfg.tba[b] * P
            eb = cfg.tbb[b] * P
            ia = np.full(ea, garb_ext % MAXI16, np.int64)
            ia[:na] = rs[a_i] % MAXI16
            if garb_ext >= MAXI16:
                ia[na:] = 0
            ib = np.full(eb, 0, np.int64)
            ib[:nb_] = rs[b_i] % MAXI16
            ixa[:, ca : ca + cfg.tba[b] * 8] = _wrap_idx(ia)
            if eb:
                ixb[:, cb : cb + cfg.tbb[b] * 8] = _wrap_idx(ib)
            off = np.full(ea + eb, -1, np.int64)
            off[:na] = slots[a_i]
            off[ea : ea + nb_] = slots[b_i]
            slotv[cfg.col0[b] * P : cfg.col0[b + 1] * P] = off
            ca += cfg.tba[b] * 8
            cb += cfg.tbb[b] * 8
        m["ixa"] = ixa
        m["ixb"] = ixb
        # one-hot tables: edge position p of tile g is slotv[g*128+p]
        sv = slotv.reshape(cfg.ttot, P)  # [g, p] (p = edge pos in tile)
        ar = np.arange(P)
        F8 = ml_dtypes.float8_e4m3fn
        # Otd[p, g*128+s] = (sv[g, p] == s)
        ot = (sv[:, :, None] == ar[None, None, :])  # [g, p, s]
        m["Otd"] = np.ascontiguousarray(
            ot.transpose(1, 0, 2).reshape(P, cfg.ttot * P).astype(F8))
        # OtTd[s, g*128+e] = (sv[g, e] == s)
        m["OtTd"] = np.ascontiguousarray(
            ot.transpose(2, 0, 1).reshape(P, cfg.ttot * P).astype(F8))

        xs = np.zeros((cfg.cap, x.shape[1]), np.float32)
        rows = (np.arange(npc) // SLOTS) * P + (np.arange(npc) % SLOTS)
        xs[rows] = x[lo : lo + npc]
        m["xT"] = _bf(xs.T)

        gsel = np.zeros((cfg.cap, G), np.float32)
        gsel[rows, np.asarray(batch[lo : lo + npc])] = 1.0
        m["gsel"] = _bf(gsel)
        maps.append(m)

    counts = np.bincount(np.asarray(batch), minlength=G).astype(np.float32)
    shared = {
        "Wl1": _bf(weights["Wl1"]),
        "Wr1": _bf(weights["Wr1"]),
        "Wl2": _bf(weights["Wl2"]),
        "Wr2": _bf(weights["Wr2"]),
        "att1r": att_rep(att1),
        "att2r": att_rep(att2),
        "b1r": _bf(np.broadcast_to(weights["b1"], (P, HC))),
        "b2r": _bf(np.broadcast_to(weights["b2"], (P, HC))),
        "ident": _bf(np.eye(P, dtype=np.float32)),
        "crecip": _f32((1.0 / np.maximum(counts, 1.0)).reshape(G, 1)),
        "Wlin1": _bf(weights["W_lin1"]),
        "blin1r": _f32(np.broadcast_to(weights["b_lin1"], (G, 64))),
        "Wout": _bf(weights["W_out"]),
        "boutr": _f32(np.full((G, 1), float(weights["b_out"][0]), np.float32)),
        "ub": _bf(u),
    }
    for m in maps:
        m.update(shared)
    return maps


# ---------------------------------------------------------------------------
# Device program
# ---------------------------------------------------------------------------


def _build(cfg: Cfg, in_dim=3):
    dt = mybir.dt
    bf = dt.bfloat16
    f32 = dt.float32
    nc = bacc.Bacc(None)
    groups = [list(range(cfg.ncores))]

    def prm(name, shape, dtype):
        return nc.declare_dram_parameter(name, list(shape), dtype, isOutput=False)

    xT = prm("xT", [in_dim, cfg.cap], bf)
    ixa = prm("ixa", [P, sum(t * 8 for t in cfg.tba)], dt.int16)
    ixb = prm("ixb", [P, max(sum(t * 8 for t in cfg.tbb), 1)], dt.int16)
    Otd = prm("Otd", [P, cfg.ttot * P], dt.float8e4)
    OtTd = prm("OtTd", [P, cfg.ttot * P], dt.float8e4)
    Wl1p = prm("Wl1", [in_dim, HC], bf)
    Wr1p = prm("Wr1", [in_dim, HC], bf)
    Wl2p = prm("Wl2", [HC, HC], bf)
    Wr2p = prm("Wr2", [HC, HC], bf)
    att1r = prm("att1r", [P, HC], bf)
    att2r = prm("att2r", [P, HC], bf)
    b1r = prm("b1r", [P, HC], bf)
    b2r = prm("b2r", [P, HC], bf)
    identp = prm("ident", [P, P], bf)
    gselp = prm("gsel", [cfg.cap, G], bf)
    crecip = prm("crecip", [G, 1], f32)
    Wlin1 = prm("Wlin1", [HC, 64], bf)
    blin1r = prm("blin1r", [G, 64], f32)
    Woutp = prm("Wout", [64 + 3, 1], bf)
    boutr = prm("boutr", [G, 1], f32)
    ub = prm("ub", [G, 3], bf)
    out_g = nc.declare_dram_parameter("out_g", [G, 1], f32, isOutput=True)

    with tile.TileContext(nc) as tc:
        with (
            tc.tile_pool(name="const", bufs=1) as constp,
            tc.tile_pool(name="meta", bufs=2) as metap,
            tc.tile_pool(name="gbuf", bufs=2) as gbufp,
            tc.tile_pool(name="work", bufs=2) as workp,
            tc.tile_pool(name="small", bufs=3) as smallp,
            tc.tile_pool(name="psA", bufs=1, space="PSUM") as psA,
            tc.tile_pool(name="psB", bufs=2, space="PSUM") as psB,
            tc.tile_pool(name="psU", bufs=2, space="PSUM") as psU,
            tc.tile_pool(name="psG", bufs=1, space="PSUM") as psG,
            tc.tile_pool(name="dram", bufs=1, space="DRAM") as dram,
        ):
            # ---- constants to SBUF ----
            def cload(p):
                t = constp.tile([p.shape[0], p.shape[1]], p.dtype, name=p.name + "_s")
                nc.sync.dma_start(out=t[:], in_=p[:])
                return t

            def cload_k(p):
                nk = (p.shape[0] + P - 1) // P
                out = []
                for kt in range(nk):
                    rows = slice(kt * P, min((kt + 1) * P, p.shape[0]))
                    t = constp.tile(
                        [rows.stop - rows.start, p.shape[1]], p.dtype,
                        name=f"{p.name}_s{kt}",
                    )
                    nc.sync.dma_start(out=t[:], in_=p[rows, :])
                    out.append(t)
                return out

            xT_s = cload(xT)
            Wl1_s = cload_k(Wl1p)
            Wr1_s = cload_k(Wr1p)
            Wl2_s = cload_k(Wl2p)
            Wr2_s = cload_k(Wr2p)
            att1r_s = cload(att1r)
            att2r_s = cload(att2r)
            b1r_s = cload(b1r)
            b2r_s = cload(b2r)
            ident_s = cload(identp)
            crecip_s = cload(crecip)
            Wlin1_s = cload_k(Wlin1)
            blin1r_s = cload(blin1r)
            Wout_s = cload(Woutp)
            boutr_s = cload(boutr)
            ub_s = cload(ub)

            # ---- internal DRAM ----
            xl1_own = dram.tile([cfg.cap, HC], bf)
            xr1_tab = dram.tile([cfg.cap, HC], bf)
            xl1_ext = dram.tile([cfg.capext, HC], bf, addr_space="Shared")
            h1T = dram.tile([2, P, cfg.cap], bf)
            xl2_own = dram.tile([cfg.cap, HC], bf)
            xr2_tab = dram.tile([cfg.cap, HC], bf)
            xl2_ext = dram.tile([cfg.capext, HC], bf, addr_space="Shared")
            gp_in = dram.tile([G, HC], f32)
            gp_out = dram.tile([G, HC], f32, addr_space="Shared")

            A_ = mybir.AluOpType
            AF = mybir.ActivationFunctionType

            # ================= node tables =================
            def node_tables(lhsT_tiles, Wl_s, Wr_s, xl_dst, xr_dst):
                for b in range(cfg.nblk):
                    rows = slice(b * P, (b + 1) * P)
                    for W_s, tab in ((Wl_s, xl_dst), (Wr_s, xr_dst)):
                        ps = psA.tile([P, HC], f32, tag="a")
                        lts = lhsT_tiles(b)
                        assert len(lts) == len(W_s)
                        for i, lt in enumerate(lts):
                            nc.tensor.matmul(
                                ps[:], lhsT=lt, rhs=W_s[i][:],
                                start=(i == 0), stop=(i == len(lts) - 1),
                            )
                        ev = smallp.tile([P, HC], bf, tag="tabev")
                        nc.scalar.activation(out=ev[:], in_=ps[:], func=AF.Copy)
                        nc.sync.dma_start(out=tab[rows, :], in_=ev[:])

            node_tables(
                lambda b: [xT_s[:, b * P : (b + 1) * P]],
                Wl1_s, Wr1_s, xl1_own, xr1_tab,
            )
            nc.gpsimd.collective_compute(
                "AllGather", A_.bypass, replica_groups=groups,
                ins=[xl1_own.opt()], outs=[xl1_ext.opt()],
            )

            # ================= edge pipeline =================
            def edge_layer(xl_ext, xr_tab, attr_s, br_s, layer):
                gpool_ps = None
                if layer == 2:
                    gpool_ps = psG.tile([G, HC], f32, name=f"gpool_ps{layer}")
                ca = cb = 0
                for b in range(cfg.nblk):
                    tb = cfg.tb[b]
                    tba, tbb = cfg.tba[b], cfg.tbb[b]
                    c0 = cfg.col0[b]
                    xr_blk = metap.tile([P, HC], bf, tag="xrblk")
                    nc.sync.dma_start(out=xr_blk[:], in_=xr_tab[b * P : (b + 1) * P, :])
                    Ot_blk = metap.tile([P, tb, P], dt.float8e4, tag="Ot")
                    nc.sync.dma_start(
                        out=Ot_blk[:].rearrange("p t s -> p (t s)"),
                        in_=Otd[:, c0 * P : (c0 + tb) * P],
                    )
                    OtT_blk = metap.tile([P, tb, P], dt.float8e4, tag="OtT")
                    nc.sync.dma_start(
                        out=OtT_blk[:].rearrange("p t e -> p (t e)"),
                        in_=OtTd[:, c0 * P : (c0 + tb) * P],
                    )
                    ixa_t = metap.tile([P, tba * 8], dt.int16, tag="ixa")
                    nc.sync.dma_start(out=ixa_t[:], in_=ixa[:, ca : ca + tba * 8])

                    CH = 5  # tiles per gather call (640 rows, HW-proven)

                    def chunked_gather(dst, dst_t0, n_tiles, table, idxt, idx_c0):
                        for q0 in range(0, n_tiles, CH):
                            q1 = min(q0 + CH, n_tiles)
                            nc.gpsimd.dma_gather(
                                out_ap=dst[:, dst_t0 + q0 : dst_t0 + q1, :],
                                in_ap=table,
                                idxs_ap=idxt[:, idx_c0 + q0 * 8 : idx_c0 + q1 * 8],
                                num_idxs=(q1 - q0) * P, num_idxs_reg=(q1 - q0) * P,
                                elem_size=HC,
                            )

                    gxl = gbufp.tile([P, tb, HC], bf, tag="gxl")
                    chunked_gather(
                        gxl, 0, tba,
                        xl_ext[0:MAXI16, :] if cfg.npiece == 2 else xl_ext[:],
                        ixa_t, 0,
                    )
                    if tbb:
                        ixb_t = metap.tile([P, tbb * 8], dt.int16, tag="ixb")
                        nc.sync.dma_start(out=ixb_t[:], in_=ixb[:, cb : cb + tbb * 8])
                        chunked_gather(
                            gxl, tba, tbb, xl_ext[MAXI16 : cfg.capext, :], ixb_t, 0
                        )

                    # u = xl[src] + xr[dst] accumulated on PE, chunk by chunk;
                    # PRelu consumes each chunk from PSUM.
                    ft = workp.tile([P, tb, HC], bf, tag="ft")
                    for q0 in range(0, tb, CT):
                        q1 = min(q0 + CT, tb)
                        ut_ps = psU.tile([P, CT, HC], f32, tag="ut")
                        for t in range(q0, q1):
                            nc.tensor.matmul(
                                ut_ps[:, t - q0, :], lhsT=OtT_blk[:, t, :],
                                rhs=xr_blk[:], start=True, stop=False,
                            )
                            nc.tensor.matmul(
                                ut_ps[:, t - q0, :], lhsT=ident_s[:],
                                rhs=gxl[:, t, :], start=False, stop=True,
                            )
                        nc.scalar.activation(
                            out=ft[:, q0:q1, :], in_=ut_ps[:, 0 : q1 - q0, :],
                            func=AF.Prelu, alpha=0.2,
                        )

                    Pt = workp.tile([P, tb, HC], bf, tag="Pt")
                    nc.vector.tensor_tensor(
                        out=Pt[:], in0=ft[:], in1=_bcast_mid(attr_s[:], tb), op=A_.mult
                    )
                    v = Pt[:].rearrange("p t (h c) -> p (t h) c", h=H)
                    t1 = workp.tile([P, tb * H, 16], bf, tag="t1")
                    nc.vector.tensor_tensor(out=t1[:], in0=v[:, :, 0:16], in1=v[:, :, 16:32], op=A_.add)
                    t2 = workp.tile([P, tb * H, 8], bf, tag="t2")
                    nc.vector.tensor_tensor(out=t2[:], in0=t1[:, :, 0:8], in1=t1[:, :, 8:16], op=A_.add)
                    t3 = workp.tile([P, tb * H, 4], bf, tag="t3")
                    nc.vector.tensor_tensor(out=t3[:], in0=t2[:, :, 0:4], in1=t2[:, :, 4:8], op=A_.add)
                    t4 = workp.tile([P, tb * H, 2], bf, tag="t4")
                    nc.vector.tensor_tensor(out=t4[:], in0=t3[:, :, 0:2], in1=t3[:, :, 2:4], op=A_.add)
                    lg = workp.tile([P, tb * H], bf, tag="lg")
                    nc.vector.tensor_tensor(
                        out=lg[:].unsqueeze(2), in0=t4[:, :, 0:1], in1=t4[:, :, 1:2], op=A_.add
                    )
                    ext = workp.tile([P, tb * H, C], bf, tag="ext")
                    nc.scalar.activation(
                        out=ext[:], in_=lg[:].to_broadcast([P, tb * H, C]), func=AF.Exp
                    )
                    msg = workp.tile([P, tb, HC + H], bf, tag="msg")
                    nc.vector.tensor_tensor(
                        out=msg[:, :, 0:HC], in0=gxl[:],
                        in1=ext[:].rearrange("p (t h) c -> p t (h c)", t=tb),
                        op=A_.mult,
                    )
                    exv = ext[:].rearrange("p (t h) c -> p t h c", t=tb)
                    nc.vector.tensor_copy(
                        out=msg[:, :, HC : HC + H], in_=exv[:, :, :, 0]
                    )

                    acc = psB.tile([P, HC + H], f32, tag="b")
                    for t in range(tb):
                        nc.tensor.matmul(
                            acc[:], lhsT=Ot_blk[:, t, :], rhs=msg[:, t, :],
                            start=(t == 0), stop=(t == tb - 1),
                        )

                    denom = smallp.tile([P, H], f32, tag="denom")
                    nc.vector.tensor_scalar(
                        out=denom[:], in0=acc[:, HC : HC + H], scalar1=1e-20,
                        scalar2=None, op0=A_.max,
                    )
                    rec = smallp.tile([P, H], f32, tag="rec")
                    nc.vector.reciprocal(out=rec[:], in_=denom[:])
                    hsc = smallp.tile([P, HC], bf, tag="hsc")
                    nc.vector.tensor_tensor(
                        out=hsc[:].rearrange("p (h c) -> p h c", h=H),
                        in0=acc[:, 0:HC].rearrange("p (h c) -> p h c", h=H),
                        in1=rec[:].to_broadcast([P, H, C]),
                        op=A_.mult,
                    )
                    hfin = smallp.tile([P, HC], bf, tag="hfin")
                    nc.vector.tensor_tensor(out=hfin[:], in0=hsc[:], in1=br_s[:], op=A_.add)
                    hout = smallp.tile([P, HC], bf, tag="hout")
                    nc.scalar.activation(out=hout[:], in_=hfin[:], func=AF.Relu)

                    if layer == 1:
                        for kt in range(2):
                            tp = psA.tile([P, P], bf, tag="a")
                            nc.tensor.transpose(
                                out=tp[:], in_=hout[:, kt * P : (kt + 1) * P],
                                identity=ident_s[:],
                            )
                            tps = smallp.tile([P, P], bf, tag="htps")
                            nc.scalar.activation(out=tps[:], in_=tp[:], func=AF.Copy)
                            nc.sync.dma_start(
                                out=h1T[kt, :, b * P : (b + 1) * P], in_=tps[:]
                            )
                    else:
                        gsel_blk = metap.tile([P, G], bf, tag="gselb")
                        nc.sync.dma_start(
                            out=gsel_blk[:], in_=gselp[b * P : (b + 1) * P, :]
                        )
                        nc.tensor.matmul(
                            gpool_ps[:], lhsT=gsel_blk[:], rhs=hout[:],
                            start=(b == 0), stop=(b == cfg.nblk - 1),
                        )
                    ca += tba * 8
                    cb += tbb * 8
                return gpool_ps

            edge_layer(xl1_ext, xr1_tab, att1r_s, b1r_s, layer=1)

            # ================= layer-2 node tables =================
            def h1_lhsT(b):
                outs = []
                for kt in range(2):
                    t = smallp.tile([P, P], bf, tag="h1l", name=f"h1l{b}_{kt}")
                    nc.sync.dma_start(out=t[:], in_=h1T[kt, :, b * P : (b + 1) * P])
                    outs.append(t[:])
                return outs

            node_tables(h1_lhsT, Wl2_s, Wr2_s, xl2_own, xr2_tab)
            nc.gpsimd.collective_compute(
                "AllGather", A_.bypass, replica_groups=groups,
                ins=[xl2_own.opt()], outs=[xl2_ext.opt()],
            )

            gpool_ps = edge_layer(xl2_ext, xr2_tab, att2r_s, b2r_s, layer=2)

            # ================= pool + MLP =================
            gsum = smallp.tile([G, HC], f32, tag="gsum")
            nc.scalar.activation(out=gsum[:], in_=gpool_ps[:], func=AF.Copy)
            nc.sync.dma_start(out=gp_in[:], in_=gsum[:])
            nc.gpsimd.collective_compute(
                "AllReduce", A_.add, replica_groups=groups,
                ins=[gp_in.opt()], outs=[gp_out.opt()],
            )
            gsum2 = smallp.tile([G, HC], f32, tag="gsum2")
            nc.sync.dma_start(out=gsum2[:], in_=gp_out[:])
            gmean = smallp.tile([G, HC], bf, tag="gmean")
            nc.vector.tensor_scalar(
                out=gmean[:], in0=gsum2[:], scalar1=crecip_s[:, 0:1], scalar2=None,
                op0=A_.mult,
            )
            gT = []
            for kt in range(2):
                tp = psA.tile([P, G], bf, tag="a")
                nc.tensor.transpose(
                    out=tp[:], in_=gmean[:, kt * P : (kt + 1) * P], identity=ident_s[:]
                )
                gkt = smallp.tile([P, G], bf, tag="gT", name=f"gT{kt}")
                nc.scalar.activation(out=gkt[:], in_=tp[:], func=AF.Copy)
                gT.append(gkt)
            lin_ps = psB.tile([G, 64], f32, tag="b")
            for kt in range(2):
                nc.tensor.matmul(
                    lin_ps[:], lhsT=gT[kt][:], rhs=Wlin1_s[kt][:],
                    start=(kt == 0), stop=(kt == 1),
                )
            lin = smallp.tile([G, 64], f32, tag="lin")
            nc.vector.tensor_tensor(out=lin[:], in0=lin_ps[:], in1=blin1r_s[:], op=A_.add)
            glu = smallp.tile([G, P], bf, tag="glu")
            nc.scalar.activation(out=glu[:, 0:64], in_=lin[:], func=AF.Relu)
            nc.vector.tensor_copy(out=glu[:, 64:67], in_=ub_s[:])
            nc.gpsimd.memset(glu[:, 67:P], 0.0)
            tp = psA.tile([P, G], bf, tag="a")
            nc.tensor.transpose(out=tp[:], in_=glu[:], identity=ident_s[:])
            gluT = smallp.tile([P, G], bf, tag="gluT")
            nc.scalar.activation(out=gluT[:], in_=tp[:], func=AF.Copy)
            out_ps = psB.tile([G, 1], f32, tag="b")
            nc.tensor.matmul(
                out_ps[:], lhsT=gluT[0:67, :], rhs=Wout_s[:], start=True, stop=True
            )
            outs = smallp.tile([G, 1], f32, tag="outs")
            nc.vector.tensor_tensor(out=outs[:], in0=out_ps[:], in1=boutr_s[:], op=A_.add)
            nc.sync.dma_start(out=out_g[:], in_=outs[:])

    nc.compile()
    _split_waits(nc)
    return nc


def _bcast_mid(ap, reps):
    return ap.unsqueeze(1).broadcast_to([ap.shape[0], reps, ap.shape[1]])


# ---------------------------------------------------------------------------
# Entry point
# ---------------------------------------------------------------------------


def kernel(**inputs):
    import os

    from concourse.bass_utils import run_bass_kernel_spmd

    x = np.asarray(inputs["x"], np.float32)
    edge_index = np.asarray(inputs["edge_index"], np.int64)
    batch = np.asarray(inputs["batch"], np.int64)
    u = np.asarray(inputs["u"], np.float32)
    weights = {
        k: np.asarray(inputs[k], np.float32)
        for k in ("Wl1", "Wr1", "att1", "b1", "Wl2", "Wr2", "att2", "b2",
                  "W_lin1", "b_lin1", "W_out", "b_out")
    }
    percore, nblk, tba, tbb = _plan_blocks(edge_index, N, NCORES)
    cfg = Cfg(N, NCORES, nblk, tba, tbb)
    maps = _prep(x, edge_index, batch, u, weights, cfg, percore)
    nc = _build(cfg, in_dim=x.shape[1])
    trace = bool(os.environ.get("KERNEL_TRACE"))
    kw = {}
    if trace:
        tmpdir = os.environ.get("KERNEL_TRACE_DIR", "/tmp/ktrace")
        os.makedirs(tmpdir, exist_ok=True)
        kw["tmpdir"] = tmpdir
    try:
        res = run_bass_kernel_spmd(nc, maps, list(range(NCORES)), trace=trace, **kw)
    except ModuleNotFoundError:
        res = run_bass_kernel_spmd(nc, maps, list(range(NCORES)))
    if trace and getattr(res, "exec_time_ns", None) is not None:
        print(f"HW exec time: {res.exec_time_ns} ns")
        if res.instructions_and_trace is not None:
            print(f"trace: {res.instructions_and_trace[1]}")
    return res.results[0]["out_g"].reshape(G).astype(np.float32)


# revision 15
# speedup vs baseline: 2.9229x; 1.2607x over previous
"""BrainAgeGAT Trainium2 kernel: 2-layer GATv2 + mean-pool + MLP on 8 NeuronCores.

Strategy (per sharding_hint: shard edges; 1D-shard nodes; all-reduce pool):
  - Edges (incl. self loops) are sorted by destination and sharded by
    destination-node range across the 8 cores, so each core owns the full
    softmax/scatter for its destination nodes.
  - Per-core destination nodes are packed into blocks of <=127 "slots"
    (slot 127 of each 128-row block is a garbage slot).
  - Node transforms xl = x@Wl / xr = x@Wr are computed on each core for its
    own node shard; the xl table is AllGather'd so every core can gather any
    source row. Per edge, 512-byte bf16 rows are fetched with dma_gather
    (SWDGE gather, int16 indices; the 51200-row global table is split in two
    halves to stay within int16).
  - Per-tile one-hot matrices (edge->slot, and transposed) are precomputed
    on the host and DMA'd from DRAM; the transposed one-hot expands per-block
    xr rows to per-edge xr via a PE matmul that accumulates with an
    identity-matmul of the gathered xl rows, yielding u = xl[src]+xr[dst]
    directly in PSUM (no per-edge xr gather, no on-device one-hot build).
  - logits = per-head tree-reduction of att * leaky_relu(u) (ACT + DVE);
    softmax needs no max subtraction at these magnitudes. The scatter
    message is exp(logit)*xl[src] (one-hot matmul accumulated in PSUM) and
    the bias is added per destination node at the end.
  - Mean-pool uses per-block one-hot graph-selector matmuls into a
    persistent PSUM accumulator, an 8-core AllReduce, and a tiny MLP.
"""

import math
import sys

sys.path.insert(0, "/opt/trn_rl_repo")

import ml_dtypes
import numpy as np

import concourse.bacc as bacc
import concourse.bass as bass
import concourse.mybir as mybir
import concourse.tile as tile
from concourse import library_config
from concourse.vector_clock import ScopedClock

BF16 = ml_dtypes.bfloat16

# ---------------------------------------------------------------------------
# Patches for walrus' one-sync-wait-per-instruction limit.
# ---------------------------------------------------------------------------


def _drain_and_barrier(self, tick_clock, wait_clock):
    nc = self.nc
    probe = nc.sync.nop(nofuse=True, hint="drain_wait_split")
    wait_clock.add_sem_waits(probe.ins, ScopedClock({None: tick_clock.global_clock}))
    si = probe.ins.sync_info
    waits = list(si.on_wait) if si and si.on_wait else []
    if len(waits) > 1:
        si.on_wait = waits[:1]
        for w in waits[1:]:
            extra = nc.sync.nop(nofuse=True, hint="drain_wait_split")
            extra.ins.sync_info = type(si)(on_wait=[w], on_update=[])
    nc.sync.drain()
    nc.all_engine_barrier()
    assert self.sems is not None
    popped = nc._tile_sem_poison_stack.pop()
    assert popped is self._sem_poison
    nc.clear_and_free_semaphores(list(self.sems.allocated().values()))
    nc.all_engine_barrier()


tile.TileContext._drain_and_barrier = _drain_and_barrier


def _split_waits(nc):
    """walrus codegen accepts one sync-wait command per instruction; Tile can
    emit several. Hoist extras onto preceding same-engine NoOps."""
    for bb in nc.main_func.blocks:
        out = []
        for ins in bb.instructions:
            si = ins.sync_info
            waits = list(si.on_wait) if si and si.on_wait else []
            if len(waits) > 1:
                for w in waits[:-1]:
                    nop = mybir.InstNoOp(
                        name=nc.get_next_instruction_name(), ins=[], outs=[]
                    )
                    nop.engine = ins.engine
                    nop.sync_info = mybir.SyncInfo(on_wait=[w], on_update=[])
                    nc.register_instruction(nop)
                    out.append(nop)
                si.on_wait = [waits[-1]]
            out.append(ins)
        bb.instructions = out


# ---------------------------------------------------------------------------
# Model dimensions (hardcoded per problem spec)
# ---------------------------------------------------------------------------
N = 50000
E = 800000
G = 128
H = 8
C = 32
HC = H * C  # 256
P = 128
NCORES = 8
SLOTS = 127  # real slots per block (slot 127 = garbage)
MAXI16 = 25600  # table-piece size for int16 gather indices
CT = 4  # tiles per PSUM u-chunk


class Cfg:
    def __init__(self, n_nodes, ncores, nblk, tba, tbb):
        self.n_nodes = n_nodes
        self.ncores = ncores
        self.nodes_pc = n_nodes // ncores
        self.nblk = nblk
        self.cap = nblk * P
        self.capext = ncores * self.cap
        self.tba = tba  # list[nblk]
        self.tbb = tbb  # list[nblk]
        self.tb = [a + b for a, b in zip(tba, tbb)]
        self.ttot = sum(self.tb)
        self.col0 = np.concatenate([[0], np.cumsum(self.tb)]).astype(int)
        self.npiece = 2 if self.capext > MAXI16 else 1
        if self.npiece == 1:
            assert all(b == 0 for b in tbb)


# ---------------------------------------------------------------------------
# Host-side preprocessing
# ---------------------------------------------------------------------------


def _f32(a):
    return np.ascontiguousarray(a, dtype=np.float32)


def _bf(a):
    return np.ascontiguousarray(np.asarray(a, dtype=np.float32).astype(BF16))


def _wrap_idx(ids):
    """Gather-index list -> [128, len/16] int16 in the SWDGE wrap layout."""
    ids = np.asarray(ids, np.int16)
    assert len(ids) % 16 == 0
    w = ids.reshape(-1, 16).T  # [16, s]
    return np.tile(w, (8, 1))  # [128, s]


def _plan_blocks(edge_index, n_nodes, ncores):
    """Sort/pad edges; return per-core edge structures + uniform tile counts."""
    npc = n_nodes // ncores
    nblk = (npc + SLOTS - 1) // SLOTS
    cap = nblk * P
    capext = ncores * cap
    npiece = 2 if capext > MAXI16 else 1

    src = np.concatenate([edge_index[0], np.arange(n_nodes)]).astype(np.int64)
    dst = np.concatenate([edge_index[1], np.arange(n_nodes)]).astype(np.int64)
    order = np.argsort(dst, kind="stable")
    src, dst = src[order], dst[order]

    sloc = src % npc
    srow = (src // npc) * cap + (sloc // SLOTS) * P + (sloc % SLOTS)

    percore = []
    na = np.zeros((ncores, nblk), int)
    nb_ = np.zeros((ncores, nblk), int)
    for c in range(ncores):
        lo = c * npc
        sel = (dst >= lo) & (dst < lo + npc)
        bsrow = srow[sel]
        loc = dst[sel] - lo
        blocks = []
        for b in range(nblk):
            es = (loc // SLOTS) == b
            rs = bsrow[es]
            slots = (loc[es] % SLOTS).astype(np.int64)
            piece = rs // MAXI16 if npiece == 2 else np.zeros_like(rs)
            a_i = np.where(piece == 0)[0]
            b_i = np.where(piece == 1)[0]
            blocks.append((rs, slots, a_i, b_i))
            na[c, b] = len(a_i)
            nb_[c, b] = len(b_i)
        percore.append(blocks)
    tba = [int(math.ceil((na[:, b].max() + 1) / P)) for b in range(nblk)]
    tbb = [int(math.ceil(nb_[:, b].max() / P)) if npiece == 2 else 0
           for b in range(nblk)]
    return percore, nblk, tba, tbb


def _prep(x, edge_index, batch, u, weights, cfg: Cfg, percore):
    npc = cfg.nodes_pc
    att1 = weights["att1"]
    att2 = weights["att2"]

    def att_rep(att):
        return _bf(np.broadcast_to(att.reshape(-1), (P, HC)))

    idx_cols_a = [t * 8 for t in cfg.tba]
    idx_cols_b = [t * 8 for t in cfg.tbb]

    maps = []
    for c in range(cfg.ncores):
        m = {}
        lo = c * npc
        ixa = np.zeros((P, sum(idx_cols_a)), np.int16)
        ixb = np.zeros((P, max(sum(idx_cols_b), 1)), np.int16)
        # per-edge slot ids in tile-major layout; -1 = pad (zero one-hot)
        slotv = np.full((cfg.ttot * P,), -1, np.int64)
        ca = cb = 0
        for b in range(cfg.nblk):
            rs, slots, a_i, b_i = percore[c][b]
            garb_ext = c * cfg.cap + b * P + 127
            na, nb_ = len(a_i), len(b_i)
            ea = cfg.tba[b] * P
            eb = cfg.tbb[b] * P
            ia = np.full(ea, garb_ext % MAXI16, np.int64)
            ia[:na] = rs[a_i] % MAXI16
            if garb_ext >= MAXI16:
                ia[na:] = 0
            ib = np.full(eb, 0, np.int64)
            ib[:nb_] = rs[b_i] % MAXI16
            ixa[:, ca : ca + cfg.tba[b] * 8] = _wrap_idx(ia)
            if eb:
                ixb[:, cb : cb + cfg.tbb[b] * 8] = _wrap_idx(ib)
            off = np.full(ea + eb, -1, np.int64)
            off[:na] = slots[a_i]
            off[ea : ea + nb_] = slots[b_i]
            slotv[cfg.col0[b] * P : cfg.col0[b + 1] * P] = off
            ca += cfg.tba[b] * 8
            cb += cfg.tbb[b] * 8
        m["ixa"] = ixa
        m["ixb"] = ixb
        # one-hot tables: edge position p of tile g is slotv[g*128+p]
        sv = slotv.reshape(cfg.ttot, P)  # [g, p] (p = edge pos in tile)
        ar = np.arange(P)
        F8 = ml_dtypes.float8_e4m3fn
        # Otd[p, g*128+s] = (sv[g, p] == s)
        ot = (sv[:, :, None] == ar[None, None, :])  # [g, p, s]
        m["Otd"] = np.ascontiguousarray(
            ot.transpose(1, 0, 2).reshape(P, cfg.ttot * P).astype(F8))
        # OtTd[s, g*128+e] = (sv[g, e] == s)
        m["OtTd"] = np.ascontiguousarray(
            ot.transpose(2, 0, 1).reshape(P, cfg.ttot * P).astype(F8))

        xs = np.zeros((cfg.cap, x.shape[1]), np.float32)
        rows = (np.arange(npc) // SLOTS) * P + (np.arange(npc) % SLOTS)
        xs[rows] = x[lo : lo + npc]
        m["xT"] = _bf(xs.T)

        gsel = np.zeros((cfg.cap, G), np.float32)
        gsel[rows, np.asarray(batch[lo : lo + npc])] = 1.0
        m["gsel"] = _bf(gsel)
        maps.append(m)

    counts = np.bincount(np.asarray(batch), minlength=G).astype(np.float32)
    shared = {
        "Wl1": _bf(weights["Wl1"]),
        "Wr1": _bf(weights["Wr1"]),
        "Wl2": _bf(weights["Wl2"]),
        "Wr2": _bf(weights["Wr2"]),
        "att1r": att_rep(att1),
        "att2r": att_rep(att2),
        "b1r": _bf(np.broadcast_to(weights["b1"], (P, HC))),
        "b2r": _bf(np.broadcast_to(weights["b2"], (P, HC))),
        "ident": _bf(np.eye(P, dtype=np.float32)),
        "crecip": _f32((1.0 / np.maximum(counts, 1.0)).reshape(G, 1)),
        "Wlin1": _bf(weights["W_lin1"]),
        "blin1r": _f32(np.broadcast_to(weights["b_lin1"], (G, 64))),
        "Wout": _bf(weights["W_out"]),
        "boutr": _f32(np.full((G, 1), float(weights["b_out"][0]), np.float32)),
        "ub": _bf(u),
    }
    for m in maps:
        m.update(shared)
    return maps


# ---------------------------------------------------------------------------
# Device program
# ---------------------------------------------------------------------------


def _build(cfg: Cfg, in_dim=3):
    dt = mybir.dt
    bf = dt.bfloat16
    f32 = dt.float32
    nc = bacc.Bacc(None)
    groups = [list(range(cfg.ncores))]

    def prm(name, shape, dtype):
        return nc.declare_dram_parameter(name, list(shape), dtype, isOutput=False)

    xT = prm("xT", [in_dim, cfg.cap], bf)
    ixa = prm("ixa", [P, sum(t * 8 for t in cfg.tba)], dt.int16)
    ixb = prm("ixb", [P, max(sum(t * 8 for t in cfg.tbb), 1)], dt.int16)
    Otd = prm("Otd", [P, cfg.ttot * P], dt.float8e4)
    OtTd = prm("OtTd", [P, cfg.ttot * P], dt.float8e4)
    Wl1p = prm("Wl1", [in_dim, HC], bf)
    Wr1p = prm("Wr1", [in_dim, HC], bf)
    Wl2p = prm("Wl2", [HC, HC], bf)
    Wr2p = prm("Wr2", [HC, HC], bf)
    att1r = prm("att1r", [P, HC], bf)
    att2r = prm("att2r", [P, HC], bf)
    b1r = prm("b1r", [P, HC], bf)
    b2r = prm("b2r", [P, HC], bf)
    identp = prm("ident", [P, P], bf)
    gselp = prm("gsel", [cfg.cap, G], bf)
    crecip = prm("crecip", [G, 1], f32)
    Wlin1 = prm("Wlin1", [HC, 64], bf)
    blin1r = prm("blin1r", [G, 64], f32)
    Woutp = prm("Wout", [64 + 3, 1], bf)
    boutr = prm("boutr", [G, 1], f32)
    ub = prm("ub", [G, 3], bf)
    out_g = nc.declare_dram_parameter("out_g", [G, 1], f32, isOutput=True)

    with tile.TileContext(nc) as tc:
        with (
            tc.tile_pool(name="const", bufs=1) as constp,
            tc.tile_pool(name="meta", bufs=3) as metap,
            tc.tile_pool(name="gbuf", bufs=2) as gbufp,
            tc.tile_pool(name="work", bufs=2) as workp,
            tc.tile_pool(name="small", bufs=3) as smallp,
            tc.tile_pool(name="psA", bufs=1, space="PSUM") as psA,
            tc.tile_pool(name="psB", bufs=2, space="PSUM") as psB,
            tc.tile_pool(name="psU", bufs=2, space="PSUM") as psU,
            tc.tile_pool(name="psG", bufs=1, space="PSUM") as psG,
            tc.tile_pool(name="dram", bufs=1, space="DRAM") as dram,
        ):
            # ---- constants to SBUF ----
            def cload(p):
                t = constp.tile([p.shape[0], p.shape[1]], p.dtype, name=p.name + "_s")
                nc.sync.dma_start(out=t[:], in_=p[:])
                return t

            def cload_k(p):
                nk = (p.shape[0] + P - 1) // P
                out = []
                for kt in range(nk):
                    rows = slice(kt * P, min((kt + 1) * P, p.shape[0]))
                    t = constp.tile(
                        [rows.stop - rows.start, p.shape[1]], p.dtype,
                        name=f"{p.name}_s{kt}",
                    )
                    nc.sync.dma_start(out=t[:], in_=p[rows, :])
                    out.append(t)
                return out

            xT_s = cload(xT)
            Wl1_s = cload_k(Wl1p)
            Wr1_s = cload_k(Wr1p)
            Wl2_s = cload_k(Wl2p)
            Wr2_s = cload_k(Wr2p)
            att1r_s = cload(att1r)
            att2r_s = cload(att2r)
            b1r_s = cload(b1r)
            b2r_s = cload(b2r)
            ident_s = cload(identp)
            crecip_s = cload(crecip)
            Wlin1_s = cload_k(Wlin1)
            blin1r_s = cload(blin1r)
            Wout_s = cload(Woutp)
            boutr_s = cload(boutr)
            ub_s = cload(ub)

            # ---- internal DRAM ----
            xl1_own = dram.tile([cfg.cap, HC], bf)
            xr1_tab = dram.tile([cfg.cap, HC], bf)
            xl1_ext = dram.tile([cfg.capext, HC], bf, addr_space="Shared")
            h1T = dram.tile([2, P, cfg.cap], bf)
            xl2_own = dram.tile([cfg.cap, HC], bf)
            xr2_tab = dram.tile([cfg.cap, HC], bf)
            xl2_ext = dram.tile([cfg.capext, HC], bf, addr_space="Shared")
            gp_in = dram.tile([G, HC], f32)
            gp_out = dram.tile([G, HC], f32, addr_space="Shared")

            A_ = mybir.AluOpType
            AF = mybir.ActivationFunctionType

            # ================= node tables =================
            def node_tables(lhsT_tiles, Wl_s, Wr_s, xl_dst, xr_dst):
                for b in range(cfg.nblk):
                    rows = slice(b * P, (b + 1) * P)
                    for W_s, tab in ((Wl_s, xl_dst), (Wr_s, xr_dst)):
                        ps = psA.tile([P, HC], f32, tag="a")
                        lts = lhsT_tiles(b)
                        assert len(lts) == len(W_s)
                        for i, lt in enumerate(lts):
                            nc.tensor.matmul(
                                ps[:], lhsT=lt, rhs=W_s[i][:],
                                start=(i == 0), stop=(i == len(lts) - 1),
                            )
                        ev = smallp.tile([P, HC], bf, tag="tabev")
                        nc.scalar.activation(out=ev[:], in_=ps[:], func=AF.Copy)
                        nc.sync.dma_start(out=tab[rows, :], in_=ev[:])

            node_tables(
                lambda b: [xT_s[:, b * P : (b + 1) * P]],
                Wl1_s, Wr1_s, xl1_own, xr1_tab,
            )
            nc.gpsimd.collective_compute(
                "AllGather", A_.bypass, replica_groups=groups,
                ins=[xl1_own.opt()], outs=[xl1_ext.opt()],
            )

            # ================= edge pipeline =================
            def edge_layer(xl_ext, xr_tab, attr_s, br_s, layer):
                gpool_ps = None
                if layer == 2:
                    gpool_ps = psG.tile([G, HC], f32, name=f"gpool_ps{layer}")
                ca = cb = 0
                for b in range(cfg.nblk):
                    tb = cfg.tb[b]
                    tba, tbb = cfg.tba[b], cfg.tbb[b]
                    c0 = cfg.col0[b]
                    xr_blk = metap.tile([P, HC], bf, tag="xrblk")
                    nc.sync.dma_start(out=xr_blk[:], in_=xr_tab[b * P : (b + 1) * P, :])
                    Ot_blk = metap.tile([P, tb, P], dt.float8e4, tag="Ot")
                    nc.sync.dma_start(
                        out=Ot_blk[:].rearrange("p t s -> p (t s)"),
                        in_=Otd[:, c0 * P : (c0 + tb) * P],
                    )
                    OtT_blk = metap.tile([P, tb, P], dt.float8e4, tag="OtT")
                    nc.sync.dma_start(
                        out=OtT_blk[:].rearrange("p t e -> p (t e)"),
                        in_=OtTd[:, c0 * P : (c0 + tb) * P],
                    )
                    ixa_t = metap.tile([P, tba * 8], dt.int16, tag="ixa")
                    nc.sync.dma_start(out=ixa_t[:], in_=ixa[:, ca : ca + tba * 8])

                    CH = 5  # tiles per gather call (640 rows, HW-proven)

                    def chunked_gather(dst, dst_t0, n_tiles, table, idxt, idx_c0):
                        for q0 in range(0, n_tiles, CH):
                            q1 = min(q0 + CH, n_tiles)
                            nc.gpsimd.dma_gather(
                                out_ap=dst[:, dst_t0 + q0 : dst_t0 + q1, :],
                                in_ap=table,
                                idxs_ap=idxt[:, idx_c0 + q0 * 8 : idx_c0 + q1 * 8],
                                num_idxs=(q1 - q0) * P, num_idxs_reg=(q1 - q0) * P,
                                elem_size=HC,
                            )

                    gxl = gbufp.tile([P, tb, HC], bf, tag="gxl", bufs=4)
                    chunked_gather(
                        gxl, 0, tba,
                        xl_ext[0:MAXI16, :] if cfg.npiece == 2 else xl_ext[:],
                        ixa_t, 0,
                    )
                    if tbb:
                        ixb_t = metap.tile([P, tbb * 8], dt.int16, tag="ixb")
                        nc.sync.dma_start(out=ixb_t[:], in_=ixb[:, cb : cb + tbb * 8])
                        chunked_gather(
                            gxl, tba, tbb, xl_ext[MAXI16 : cfg.capext, :], ixb_t, 0
                        )

                    # u = xl[src] + xr[dst] accumulated on PE, chunk by chunk;
                    # PRelu consumes each chunk from PSUM.
                    ft = workp.tile([P, tb, HC], bf, tag="ft")
                    for q0 in range(0, tb, CT):
                        q1 = min(q0 + CT, tb)
                        ut_ps = psU.tile([P, CT, HC], f32, tag="ut")
                        for t in range(q0, q1):
                            nc.tensor.matmul(
                                ut_ps[:, t - q0, :], lhsT=OtT_blk[:, t, :],
                                rhs=xr_blk[:], start=True, stop=False,
                            )
                            nc.tensor.matmul(
                                ut_ps[:, t - q0, :], lhsT=ident_s[:],
                                rhs=gxl[:, t, :], start=False, stop=True,
                            )
                        nc.scalar.activation(
                            out=ft[:, q0:q1, :], in_=ut_ps[:, 0 : q1 - q0, :],
                            func=AF.Prelu, alpha=0.2,
                        )

                    Pt = workp.tile([P, tb, HC], bf, tag="Pt")
                    nc.vector.tensor_tensor(
                        out=Pt[:], in0=ft[:], in1=_bcast_mid(attr_s[:], tb), op=A_.mult
                    )
                    v = Pt[:].rearrange("p t (h c) -> p (t h) c", h=H)
                    t1 = workp.tile([P, tb * H, 16], bf, tag="t1")
                    nc.vector.tensor_tensor(out=t1[:], in0=v[:, :, 0:16], in1=v[:, :, 16:32], op=A_.add)
                    t2 = workp.tile([P, tb * H, 8], bf, tag="t2")
                    nc.vector.tensor_tensor(out=t2[:], in0=t1[:, :, 0:8], in1=t1[:, :, 8:16], op=A_.add)
                    t3 = workp.tile([P, tb * H, 4], bf, tag="t3")
                    nc.vector.tensor_tensor(out=t3[:], in0=t2[:, :, 0:4], in1=t2[:, :, 4:8], op=A_.add)
                    t4 = workp.tile([P, tb * H, 2], bf, tag="t4")
                    nc.vector.tensor_tensor(out=t4[:], in0=t3[:, :, 0:2], in1=t3[:, :, 2:4], op=A_.add)
                    lg = workp.tile([P, tb * H], bf, tag="lg")
                    nc.vector.tensor_tensor(
                        out=lg[:].unsqueeze(2), in0=t4[:, :, 0:1], in1=t4[:, :, 1:2], op=A_.add
                    )
                    ext = workp.tile([P, tb * H, C], bf, tag="ext")
                    nc.scalar.activation(
                        out=ext[:], in_=lg[:].to_broadcast([P, tb * H, C]), func=AF.Exp
                    )
                    msg = workp.tile([P, tb, HC + H], bf, tag="msg")
                    nc.vector.tensor_tensor(
                        out=msg[:, :, 0:HC], in0=gxl[:],
                        in1=ext[:].rearrange("p (t h) c -> p t (h c)", t=tb),
                        op=A_.mult,
                    )
                    exv = ext[:].rearrange("p (t h) c -> p t h c", t=tb)
                    nc.vector.tensor_copy(
                        out=msg[:, :, HC : HC + H], in_=exv[:, :, :, 0]
                    )

                    acc = psB.tile([P, HC + H], f32, tag="b")
                    for t in range(tb):
                        nc.tensor.matmul(
                            acc[:], lhsT=Ot_blk[:, t, :], rhs=msg[:, t, :],
                            start=(t == 0), stop=(t == tb - 1),
                        )

                    denom = smallp.tile([P, H], f32, tag="denom")
                    nc.vector.tensor_scalar(
                        out=denom[:], in0=acc[:, HC : HC + H], scalar1=1e-20,
                        scalar2=None, op0=A_.max,
                    )
                    rec = smallp.tile([P, H], f32, tag="rec")
                    nc.vector.reciprocal(out=rec[:], in_=denom[:])
                    hsc = smallp.tile([P, HC], bf, tag="hsc")
                    nc.vector.tensor_tensor(
                        out=hsc[:].rearrange("p (h c) -> p h c", h=H),
                        in0=acc[:, 0:HC].rearrange("p (h c) -> p h c", h=H),
                        in1=rec[:].to_broadcast([P, H, C]),
                        op=A_.mult,
                    )
                    hfin = smallp.tile([P, HC], bf, tag="hfin")
                    nc.vector.tensor_tensor(out=hfin[:], in0=hsc[:], in1=br_s[:], op=A_.add)
                    hout = smallp.tile([P, HC], bf, tag="hout")
                    nc.scalar.activation(out=hout[:], in_=hfin[:], func=AF.Relu)

                    if layer == 1:
                        for kt in range(2):
                            tp = psA.tile([P, P], bf, tag="a")
                            nc.tensor.transpose(
                                out=tp[:], in_=hout[:, kt * P : (kt + 1) * P],
                                identity=ident_s[:],
                            )
                            tps = smallp.tile([P, P], bf, tag="htps")
                            nc.scalar.activation(out=tps[:], in_=tp[:], func=AF.Copy)
                            nc.sync.dma_start(
                                out=h1T[kt, :, b * P : (b + 1) * P], in_=tps[:]
                            )
                    else:
                        gsel_blk = metap.tile([P, G], bf, tag="gselb")
                        nc.sync.dma_start(
                            out=gsel_blk[:], in_=gselp[b * P : (b + 1) * P, :]
                        )
                        nc.tensor.matmul(
                            gpool_ps[:], lhsT=gsel_blk[:], rhs=hout[:],
                            start=(b == 0), stop=(b == cfg.nblk - 1),
                        )
                    ca += tba * 8
                    cb += tbb * 8
                return gpool_ps

            edge_layer(xl1_ext, xr1_tab, att1r_s, b1r_s, layer=1)

            # ================= layer-2 node tables =================
            def h1_lhsT(b):
                outs = []
                for kt in range(2):
                    t = smallp.tile([P, P], bf, tag="h1l", name=f"h1l{b}_{kt}")
                    nc.sync.dma_start(out=t[:], in_=h1T[kt, :, b * P : (b + 1) * P])
                    outs.append(t[:])
                return outs

            node_tables(h1_lhsT, Wl2_s, Wr2_s, xl2_own, xr2_tab)
            nc.gpsimd.collective_compute(
                "AllGather", A_.bypass, replica_groups=groups,
                ins=[xl2_own.opt()], outs=[xl2_ext.opt()],
            )

            gpool_ps = edge_layer(xl2_ext, xr2_tab, att2r_s, b2r_s, layer=2)

            # ================= pool + MLP =================
            gsum = smallp.tile([G, HC], f32, tag="gsum")
            nc.scalar.activation(out=gsum[:], in_=gpool_ps[:], func=AF.Copy)
            nc.sync.dma_start(out=gp_in[:], in_=gsum[:])
            nc.gpsimd.collective_compute(
                "AllReduce", A_.add, replica_groups=groups,
                ins=[gp_in.opt()], outs=[gp_out.opt()],
            )
            gsum2 = smallp.tile([G, HC], f32, tag="gsum2")
            nc.sync.dma_start(out=gsum2[:], in_=gp_out[:])
            gmean = smallp.tile([G, HC], bf, tag="gmean")
            nc.vector.tensor_scalar(
                out=gmean[:], in0=gsum2[:], scalar1=crecip_s[:, 0:1], scalar2=None,
                op0=A_.mult,
            )
            gT = []
            for kt in range(2):
                tp = psA.tile([P, G], bf, tag="a")
                nc.tensor.transpose(
                    out=tp[:], in_=gmean[:, kt * P : (kt + 1) * P], identity=ident_s[:]
                )
                gkt = smallp.tile([P, G], bf, tag="gT", name=f"gT{kt}")
                nc.scalar.activation(out=gkt[:], in_=tp[:], func=AF.Copy)
                gT.append(gkt)
            lin_ps = psB.tile([G, 64], f32, tag="b")
            for kt in range(2):
                nc.tensor.matmul(
                    lin_ps[:], lhsT=gT[kt][:], rhs=Wlin1_s[kt][:],
                    start=(kt == 0), stop=(kt == 1),
                )
            lin = smallp.tile([G, 64], f32, tag="lin")
            nc.vector.tensor_tensor(out=lin[:], in0=lin_ps[:], in1=blin1r_s[:], op=A_.add)
            glu = smallp.tile([G, P], bf, tag="glu")
            nc.scalar.activation(out=glu[:, 0:64], in_=lin[:], func=AF.Relu)
            nc.vector.tensor_copy(out=glu[:, 64:67], in_=ub_s[:])
            nc.gpsimd.memset(glu[:, 67:P], 0.0)
            tp = psA.tile([P, G], bf, tag="a")
            nc.tensor.transpose(out=tp[:], in_=glu[:], identity=ident_s[:])
            gluT = smallp.tile([P, G], bf, tag="gluT")
            nc.scalar.activation(out=gluT[:], in_=tp[:], func=AF.Copy)
            out_ps = psB.tile([G, 1], f32, tag="b")
            nc.tensor.matmul(
                out_ps[:], lhsT=gluT[0:67, :], rhs=Wout_s[:], start=True, stop=True
            )
            outs = smallp.tile([G, 1], f32, tag="outs")
            nc.vector.tensor_tensor(out=outs[:], in0=out_ps[:], in1=boutr_s[:], op=A_.add)
            nc.sync.dma_start(out=out_g[:], in_=outs[:])

    nc.compile()
    _split_waits(nc)
    return nc


def _bcast_mid(ap, reps):
    return ap.unsqueeze(1).broadcast_to([ap.shape[0], reps, ap.shape[1]])


# ---------------------------------------------------------------------------
# Entry point
# ---------------------------------------------------------------------------


def kernel(**inputs):
    import os

    from concourse.bass_utils import run_bass_kernel_spmd

    x = np.asarray(inputs["x"], np.float32)
    edge_index = np.asarray(inputs["edge_index"], np.int64)
    batch = np.asarray(inputs["batch"], np.int64)
    u = np.asarray(inputs["u"], np.float32)
    weights = {
        k: np.asarray(inputs[k], np.float32)
        for k in ("Wl1", "Wr1", "att1", "b1", "Wl2", "Wr2", "att2", "b2",
                  "W_lin1", "b_lin1", "W_out", "b_out")
    }
    percore, nblk, tba, tbb = _plan_blocks(edge_index, N, NCORES)
    cfg = Cfg(N, NCORES, nblk, tba, tbb)
    maps = _prep(x, edge_index, batch, u, weights, cfg, percore)
    nc = _build(cfg, in_dim=x.shape[1])
    trace = bool(os.environ.get("KERNEL_TRACE"))
    kw = {}
    if trace:
        tmpdir = os.environ.get("KERNEL_TRACE_DIR", "/tmp/ktrace")
        os.makedirs(tmpdir, exist_ok=True)
        kw["tmpdir"] = tmpdir
    try:
        res = run_bass_kernel_spmd(nc, maps, list(range(NCORES)), trace=trace, **kw)
    except ModuleNotFoundError:
        res = run_bass_kernel_spmd(nc, maps, list(range(NCORES)))
    if trace and getattr(res, "exec_time_ns", None) is not None:
        print(f"HW exec time: {res.exec_time_ns} ns")
        if res.instructions_and_trace is not None:
            print(f"trace: {res.instructions_and_trace[1]}")
    return res.results[0]["out_g"].reshape(G).astype(np.float32)
